# revision 1
# baseline (speedup 1.0000x reference)
"""ConcatCritic pair-grid MLP on 8 TRN2 NeuronCores.

Computes out[i, j] = f(x[i], y[j]) where f is a 3-hidden-layer MLP over the
concatenated pair, decomposed so the first layer is two small projections
summed by broadcast (no [B, B, A+B] concat tensor).

Sharding: the B^2 pair grid is split row-wise (x batch) across 8 cores;
y and all MLP parameters are replicated. Each core produces a [B/8, B]
score tile; the host concatenates them. b3 (a scalar) is added on the host.

Device layout: activations live transposed as [hid-on-partitions, pairs-on-
free] so every layer matmul is lhsT=W_block [128(k),128(m)], rhs=hT
[128(k), 512(pairs)] accumulating over 4 k-blocks into PSUM. Matmul operands
are float32r (fp22 multiply at full PE rate, fp32 accumulate). PSUM->SBUF
relu+bias drains are split between ScalarE and VectorE to keep both under
the TensorE span.
"""

import numpy as np

import concourse.bass as bass
import concourse.mybir as mybir
from concourse import bacc
from concourse.bass_utils import run_bass_kernel_spmd
from concourse.tile import TileContext

B = 256
A_DIM = 128
HID = 512
N_CORES = 8
ROWS = B // N_CORES  # 32 x-rows per core
KB = HID // 128  # 4 k-blocks of 128
PAIR_TILE = 512  # pairs per matmul tile = 2 x-rows x 256 y-rows
ROWS_PER_TILE = PAIR_TILE // B  # 2
N_TILES = ROWS // ROWS_PER_TILE  # 16

F32 = mybir.dt.float32
F32R = mybir.dt.float32r

_CACHE = {}


def _build_nc():
    nc = bacc.Bacc()

    xT = nc.declare_dram_parameter("xT", [A_DIM, ROWS], F32R, isOutput=False)
    yT = nc.declare_dram_parameter("yT", [A_DIM, B], F32R, isOutput=False)
    Wx = nc.declare_dram_parameter("Wx", [A_DIM, HID], F32R, isOutput=False)
    Wy = nc.declare_dram_parameter("Wy", [A_DIM, HID], F32R, isOutput=False)
    W1 = nc.declare_dram_parameter("W1", [HID, HID], F32R, isOutput=False)
    W2 = nc.declare_dram_parameter("W2", [HID, HID], F32R, isOutput=False)
    W3 = nc.declare_dram_parameter("W3", [HID, 1], F32R, isOutput=False)
    b0r = nc.declare_dram_parameter("b0r", [128, KB], F32, isOutput=False)
    b1r = nc.declare_dram_parameter("b1r", [128, KB], F32, isOutput=False)
    b2r = nc.declare_dram_parameter("b2r", [128, KB], F32, isOutput=False)
    out = nc.declare_dram_parameter("out", [1, ROWS * B], F32, isOutput=True)

    relu = mybir.ActivationFunctionType.Relu

    with TileContext(nc) as tc:
        with (
            tc.tile_pool(name="const", bufs=1) as const,
            tc.tile_pool(name="work", bufs=3) as work,
            tc.tile_pool(name="sc_pool", bufs=4) as sc_pool,
            tc.tile_pool(name="psum", bufs=6, space="PSUM") as psum,
            tc.tile_pool(name="psum_s", bufs=2, space="PSUM") as psum_s,
        ):
            # ---- load replicated constants -------------------------------
            xT_sb = const.tile([A_DIM, ROWS], F32R)
            yT_sb = const.tile([A_DIM, B], F32R)
            Wx_sb = const.tile([A_DIM, HID], F32R)
            Wy_sb = const.tile([A_DIM, HID], F32R)
            b0_sb = const.tile([128, KB], F32)
            b1_sb = const.tile([128, KB], F32)
            b2_sb = const.tile([128, KB], F32)
            W1_sb = const.tile([128, KB, HID], F32R)
            W2_sb = const.tile([128, KB, HID], F32R)
            W3_sb = const.tile([128, KB, 1], F32R)

            nc.sync.dma_start(xT_sb[:], xT[:, :])
            nc.sync.dma_start(Wx_sb[:], Wx[:, :])
            nc.sync.dma_start(yT_sb[:], yT[:, :])
            nc.sync.dma_start(Wy_sb[:], Wy[:, :])
            nc.sync.dma_start(b0_sb[:], b0r[:, :])
            # W1 chunks before anything L2 needs: tile-0 layer-1 k-group
            # matmuls gate on W1 k-block arrival.
            w1_r = W1[:, :].rearrange("(k p) n -> p k n", p=128)
            w2_r = W2[:, :].rearrange("(k p) n -> p k n", p=128)
            for k in range(KB):
                nc.sync.dma_start(W1_sb[:, k], w1_r[:, k])
            nc.sync.dma_start(b1_sb[:], b1r[:, :])
            for k in range(KB):
                nc.sync.dma_start(W2_sb[:, k], w2_r[:, k])
            nc.sync.dma_start(b2_sb[:], b2r[:, :])
            nc.sync.dma_start(W3_sb[:], W3[:, :].rearrange("(k p) n -> p k n", p=128))

            # ---- input projections --------------------------------------
            # bxT[p, m, i] = (x @ Wx)^T[m*128+p, i] + b0[m*128+p]
            # hx/hy interleaved per block m and drains split DVE/ACT so the
            # first pair-tile's layer-0 (DVE) and layer-1 (PE) start early.
            bxT = const.tile([128, KB, ROWS], F32)
            hyT = const.tile([128, KB, B], F32)
            for m in range(KB):
                sl = slice(m * 128, (m + 1) * 128)
                ph = psum.tile([128, PAIR_TILE], F32, tag="ps", name="ph")[:, :ROWS]
                nc.tensor.matmul(ph, Wx_sb[:, sl], xT_sb[:], start=True, stop=True)
                nc.vector.tensor_scalar_add(bxT[:, m], ph, b0_sb[:, m : m + 1])
                ph2 = psum.tile([128, PAIR_TILE], F32, tag="ps", name="ph2")[:, :B]
                nc.tensor.matmul(ph2, Wy_sb[:, sl], yT_sb[:], start=True, stop=True)
                nc.scalar.copy(out=hyT[:, m], in_=ph2)

            # ---- main pair-tile loop ------------------------------------
            for t in range(N_TILES):
                i0 = t * ROWS_PER_TILE
                # layer 0 on DVE (SBUF->SBUF is cheap there):
                # h0T[p, k, a*256+j] = relu(hyT[p,k,j] + bxT[p,k,i0+a])
                h0T = work.tile([128, KB, PAIR_TILE], F32R, tag="h0")
                for k in range(KB):
                    for a in range(ROWS_PER_TILE):
                        nc.vector.tensor_scalar(
                            h0T[:, k, a * B : (a + 1) * B],
                            hyT[:, k],
                            bxT[:, k, i0 + a : i0 + a + 1],
                            0.0,
                            mybir.AluOpType.add,
                            mybir.AluOpType.max,
                        )
                # layers 1 and 2; PSUM drains (relu+bias) split 5:3 between
                # ScalarE and VectorE so both stay under the TensorE span.
                hin = h0T
                for layer, (W_sb, b_sb) in enumerate(((W1_sb, b1_sb), (W2_sb, b2_sb))):
                    hout = work.tile([128, KB, PAIR_TILE], F32R, tag=f"h{layer + 1}")
                    for m in range(KB):
                        pt = psum.tile([128, PAIR_TILE], F32, tag="ps", name="pt")
                        for k in range(KB):
                            nc.tensor.matmul(
                                pt,
                                W_sb[:, k, m * 128 : (m + 1) * 128],
                                hin[:, k],
                                start=(k == 0),
                                stop=(k == KB - 1),
                            )
                        on_act = (m % 2 == 0) if layer == 0 else (m != 3)
                        if on_act:
                            nc.scalar.activation(
                                hout[:, m],
                                pt,
                                relu,
                                bias=b_sb[:, m : m + 1],
                                scale=1.0,
                            )
                        else:
                            nc.vector.tensor_scalar(
                                hout[:, m],
                                pt,
                                b_sb[:, m : m + 1],
                                0.0,
                                mybir.AluOpType.add,
                                mybir.AluOpType.max,
                            )
                    hin = hout
                # layer 3: [1, 512] scores for this tile (b3 added on host)
                ps = psum_s.tile([128, PAIR_TILE], F32, tag="sc", name="ps")[:1]
                for k in range(KB):
                    nc.tensor.matmul(
                        ps,
                        W3_sb[:, k],
                        hin[:, k],
                        start=(k == 0),
                        stop=(k == KB - 1),
                    )
                sc_sb = sc_pool.tile([1, PAIR_TILE], F32, tag="sc_sb")
                nc.scalar.copy(out=sc_sb[:], in_=ps)
                nc.sync.dma_start(
                    out[:, t * PAIR_TILE : (t + 1) * PAIR_TILE], sc_sb[:]
                )

    nc.compile()
    return nc


def _get_nc():
    if "nc" not in _CACHE:
        _CACHE["nc"] = _build_nc()
    return _CACHE["nc"]


def _prep_in_maps(inputs):
    f = lambda a: np.ascontiguousarray(np.asarray(a), dtype=np.float32)
    x, y = f(inputs["x"]), f(inputs["y"])
    shared = {
        "yT": f(y.T),
        "Wx": f(inputs["Wx"]),
        "Wy": f(inputs["Wy"]),
        "W1": f(inputs["W1"]),
        "W2": f(inputs["W2"]),
        "W3": f(inputs["W3"]),
        "b0r": f(np.asarray(inputs["b0"]).reshape(KB, 128).T),
        "b1r": f(np.asarray(inputs["b1"]).reshape(KB, 128).T),
        "b2r": f(np.asarray(inputs["b2"]).reshape(KB, 128).T),
    }
    in_maps = []
    for m in range(N_CORES):
        im = dict(shared)
        im["xT"] = f(x[m * ROWS : (m + 1) * ROWS].T)
        in_maps.append(im)
    return in_maps


def run(trace=False, **inputs):
    nc = _get_nc()
    in_maps = _prep_in_maps(inputs)
    res = run_bass_kernel_spmd(nc, in_maps, core_ids=list(range(N_CORES)), trace=trace)
    b3 = np.float32(np.asarray(inputs["b3"]).reshape(-1)[0])
    blocks = [r["out"].reshape(ROWS, B) + b3 for r in res.results]
    return np.concatenate(blocks, axis=0).astype(np.float32), res


def kernel(**inputs):
    out, _ = run(trace=False, **inputs)
    return out



# revision 5
# speedup vs baseline: 1.8366x; 1.8366x over previous
"""ConcatCritic pair-grid MLP on 8 TRN2 NeuronCores — fp8 DoubleRow version.

Computes out[i, j] = f(x[i], y[j]) where f is a 3-hidden-layer MLP over the
concatenated pair, decomposed so the first layer is two small projections
summed by broadcast (no [B, B, A+B] concat tensor).

Sharding: the B^2 pair grid is split row-wise (x batch) across 8 cores;
y and all MLP parameters are replicated. Each core produces a [B/8, B]
score tile; the host concatenates them.

Precision scheme (validated in numpy, rel err ~1.1e-2 vs 2e-2 gate):
- Input projections hx, hy in exact fp32r, pre-scaled by S_H on the host.
- h0 = relu(hx + hy + b0) stored as e4m3 at scale S_H (DVE tensor_scalar,
  SBUF->SBUF so it runs in the 2x_2p DVE fast mode).
- Layers 1 and 2 run as fp8 e4m3 DoubleRow matmuls (2 k-blocks of 128 per
  instruction, 0.5 cycles/row): weights quantized at scale S_W with an fp8
  RESIDUAL tensor (W = Wq + Wr, both e4m3) so weight quantization noise is
  cancelled; activations carry ~0.8% noise each, which is the remaining
  error. psum = S_H*S_W*z.
- L1 drain must rescale by 1/S_W to re-quantize h1 to e4m3: ScalarE
  activation(relu, bias, scale). L2 drain keeps the S_H*S_W scale in fp32
  (no rescale needed -> runs as add+relu on DVE/GpSimd/ScalarE).
- Layer 3 uses the stationary-operand trick: lhsT = h2 128-pair block
  (fp32r), rhs = W3 k-column [128,1], out [128 pairs, 1] -- the cost model
  charges by moving size (1), so L3 is nearly free on PE. k-blocks
  accumulate into a single psum column per pair block (HW-exact psum
  group). Final scale 1/(S_H*S_W) and +b3 applied on the host.
"""

import numpy as np
import ml_dtypes

import concourse.bass as bass
import concourse.mybir as mybir
from concourse import bacc
from concourse.bass_utils import run_bass_kernel_spmd
from concourse.tile import TileContext

B = 256
A_DIM = 128
HID = 512
N_CORES = 8
ROWS = B // N_CORES  # 32 x-rows per core
KB = HID // 128  # 4 k-blocks of 128
PAIR_TILE = 512  # pairs per tile = 2 x-rows x 256 y-rows
ROWS_PER_TILE = PAIR_TILE // B  # 2
N_TILES = ROWS // ROWS_PER_TILE  # 16

S_H = 64.0  # activation fp8 scale
S_W = 2048.0  # weight fp8 scale

RES1 = True  # W1 residual compensation
RES2 = True  # W2 residual compensation

F32 = mybir.dt.float32
F32R = mybir.dt.float32r
F8 = mybir.dt.float8e4
E4 = ml_dtypes.float8_e4m3
DR = mybir.MatmulPerfMode.DoubleRow

_CACHE = {}


def _build_nc():
    nc = bacc.Bacc()

    xT = nc.declare_dram_parameter("xT", [A_DIM, ROWS], F32R, isOutput=False)
    yT = nc.declare_dram_parameter("yT", [A_DIM, B], F32R, isOutput=False)
    Wxs = nc.declare_dram_parameter("Wxs", [A_DIM, HID], F32R, isOutput=False)
    Wys = nc.declare_dram_parameter("Wys", [A_DIM, HID], F32R, isOutput=False)
    b0s = nc.declare_dram_parameter("b0s", [128, KB], F32, isOutput=False)
    W1q = nc.declare_dram_parameter("W1q", [128, KB, HID], F8, isOutput=False)
    W1r = nc.declare_dram_parameter("W1r", [128, KB, HID], F8, isOutput=False)
    W2q = nc.declare_dram_parameter("W2q", [128, KB, HID], F8, isOutput=False)
    W2r = nc.declare_dram_parameter("W2r", [128, KB, HID], F8, isOutput=False)
    b1s = nc.declare_dram_parameter("b1s", [128, KB], F32, isOutput=False)
    b2s = nc.declare_dram_parameter("b2s", [128, KB], F32, isOutput=False)
    W3c = nc.declare_dram_parameter("W3c", [128, KB, 2], F32R, isOutput=False)
    out = nc.declare_dram_parameter("out", [128, N_TILES * 4], F32, isOutput=True)

    relu = mybir.ActivationFunctionType.Relu
    ADD = mybir.AluOpType.add
    MAX = mybir.AluOpType.max

    with TileContext(nc) as tc:
        with (
            tc.tile_pool(name="const", bufs=1) as const,
            tc.tile_pool(name="work", bufs=3) as work,
            tc.tile_pool(name="sc_pool", bufs=2) as sc_pool,
            tc.tile_pool(name="ps1", bufs=3, space="PSUM") as ps1,
            tc.tile_pool(name="ps2", bufs=3, space="PSUM") as ps2,
            tc.tile_pool(name="ps3", bufs=2, space="PSUM") as ps3,
        ):
            # ---- load replicated constants -------------------------------
            xT_sb = const.tile([A_DIM, ROWS], F32R)
            yT_sb = const.tile([A_DIM, B], F32R)
            Wxs_sb = const.tile([A_DIM, HID], F32R)
            Wys_sb = const.tile([A_DIM, HID], F32R)
            b0_sb = const.tile([128, KB], F32)
            b1_sb = const.tile([128, KB], F32)
            b2_sb = const.tile([128, KB], F32)
            W1q_sb = const.tile([128, KB, HID], F8)
            W1r_sb = const.tile([128, KB, HID], F8)
            W2q_sb = const.tile([128, KB, HID], F8)
            W2r_sb = const.tile([128, KB, HID], F8)
            W3_sb = const.tile([128, KB, 2], F32R)

            nc.sync.dma_start(xT_sb[:], xT[:, :])
            nc.sync.dma_start(Wxs_sb[:], Wxs[:, :])
            nc.sync.dma_start(yT_sb[:], yT[:, :])
            nc.sync.dma_start(Wys_sb[:], Wys[:, :])
            nc.sync.dma_start(b0_sb[:], b0s[:, :])
            nc.sync.dma_start(W1q_sb[:], W1q[:, :, :])
            nc.sync.dma_start(W1r_sb[:], W1r[:, :, :])
            nc.sync.dma_start(b1_sb[:], b1s[:, :])
            nc.sync.dma_start(W2q_sb[:], W2q[:, :, :])
            nc.sync.dma_start(W2r_sb[:], W2r[:, :, :])
            nc.sync.dma_start(b2_sb[:], b2s[:, :])
            nc.sync.dma_start(W3_sb[:], W3c[:, :, :])

            # ---- input projections (exact fp32r, pre-scaled by S_H) ------
            bxT = const.tile([128, KB, ROWS], F32)
            hyT = const.tile([128, KB, B], F32)
            for m in range(KB):
                sl = slice(m * 128, (m + 1) * 128)
                ph = ps1.tile([128, PAIR_TILE], F32, tag="ps1", name="ph")[:, :ROWS]
                nc.tensor.matmul(ph, Wxs_sb[:, sl], xT_sb[:], start=True, stop=True)
                nc.vector.tensor_scalar(
                    bxT[:, m], ph, b0_sb[:, m : m + 1], None, ADD
                )
                ph2 = ps2.tile([128, PAIR_TILE], F32, tag="ps2", name="ph2")[:, :B]
                nc.tensor.matmul(ph2, Wys_sb[:, sl], yT_sb[:], start=True, stop=True)
                nc.scalar.copy(out=hyT[:, m], in_=ph2)

            # ---- main pair-tile loop ------------------------------------
            sc_sb = None
            for t in range(N_TILES):
                i0 = t * ROWS_PER_TILE
                # layer 0: h0q = e4m3(S_H * relu(hx_i + hy_j + b0))
                # (hx, b0 pre-scaled into bxT; hy pre-scaled into hyT)
                h0q = work.tile([128, KB, PAIR_TILE], F8, tag="h0")
                for k in range(KB):
                    # GPSIMD is SBUF-only, so it takes half of layer 0 here
                    # (its only PSUM-free work); DVE takes the other half.
                    eng = nc.gpsimd if k < 2 else nc.vector
                    for a in range(ROWS_PER_TILE):
                        eng.tensor_scalar(
                            h0q[:, k, a * B : (a + 1) * B],
                            hyT[:, k],
                            bxT[:, k, i0 + a : i0 + a + 1],
                            0.0,
                            ADD,
                            MAX,
                        )
                # layer 1: fp8 DoubleRow + weight residual -> ACT rescale drain
                h1q = work.tile([128, KB, PAIR_TILE], F8, tag="h1")
                for m in range(KB):
                    mc = slice(m * 128, (m + 1) * 128)
                    pt = ps1.tile([128, PAIR_TILE], F32, tag="ps1", name="pt1")
                    nc.tensor.matmul(
                        pt, W1q_sb[:, 0:2, mc], h0q[:, 0:2, :],
                        start=True, stop=False, perf_mode=DR,
                    )
                    nc.tensor.matmul(
                        pt, W1q_sb[:, 2:4, mc], h0q[:, 2:4, :],
                        start=False, stop=not RES1, perf_mode=DR,
                    )
                    if RES1:
                        nc.tensor.matmul(
                            pt, W1r_sb[:, 0:2, mc], h0q[:, 0:2, :],
                            start=False, stop=False, perf_mode=DR,
                        )
                        nc.tensor.matmul(
                            pt, W1r_sb[:, 2:4, mc], h0q[:, 2:4, :],
                            start=False, stop=True, perf_mode=DR,
                        )
                    nc.scalar.activation(
                        h1q[:, m], pt, relu,
                        bias=b1_sb[:, m : m + 1], scale=1.0 / S_W,
                    )
                # layer 2: fp8 DoubleRow + residual -> scale-free drains
                # (h2 stays fp32 at S_H*S_W scale; engines: DVE/Pool/ACT mix)
                h2 = work.tile([128, KB, PAIR_TILE], F32R, tag="h2")
                for m in range(KB):
                    mc = slice(m * 128, (m + 1) * 128)
                    pt = ps2.tile([128, PAIR_TILE], F32, tag="ps2", name="pt2")
                    nc.tensor.matmul(
                        pt, W2q_sb[:, 0:2, mc], h1q[:, 0:2, :],
                        start=True, stop=False, perf_mode=DR,
                    )
                    nc.tensor.matmul(
                        pt, W2q_sb[:, 2:4, mc], h1q[:, 2:4, :],
                        start=False, stop=not RES2, perf_mode=DR,
                    )
                    if RES2:
                        nc.tensor.matmul(
                            pt, W2r_sb[:, 0:2, mc], h1q[:, 0:2, :],
                            start=False, stop=False, perf_mode=DR,
                        )
                        nc.tensor.matmul(
                            pt, W2r_sb[:, 2:4, mc], h1q[:, 2:4, :],
                            start=False, stop=True, perf_mode=DR,
                        )
                    if m == 0:
                        nc.scalar.activation(
                            h2[:, m], pt, relu,
                            bias=b2_sb[:, m : m + 1], scale=1.0,
                        )
                    else:
                        nc.vector.tensor_scalar(
                            h2[:, m], pt, b2_sb[:, m : m + 1], 0.0, ADD, MAX
                        )
                # layer 3: stationary-h2 trick; one psum column per 128-pair
                # block, k-accumulated (HW-exact group), tiny DVE drains.
                if t % 4 == 0:
                    sc_sb = sc_pool.tile([128, 16], F32, tag="sc")
                for pb in range(4):
                    pc = slice(pb * 128, (pb + 1) * 128)
                    # fp32r ISA needs even moving/dst free sizes, so W3
                    # is duplicated to 2 identical columns; col 0 is drained.
                    p3 = ps3.tile([128, PAIR_TILE], F32, tag="ps3", name="p3")[:, :2]
                    for k in range(KB):
                        nc.tensor.matmul(
                            p3, h2[:, k, pc], W3_sb[:, k],
                            start=(k == 0), stop=(k == KB - 1),
                        )
                    col = (t % 4) * 4 + pb
                    nc.vector.tensor_scalar(
                        sc_sb[:, col : col + 1], p3[:, 0:1], 0.0, None, ADD
                    )
                if t % 4 == 3:
                    g = t // 4
                    nc.sync.dma_start(out[:, g * 16 : (g + 1) * 16], sc_sb[:])

    nc.compile()
    return nc


def _get_nc():
    if "nc" not in _CACHE:
        _CACHE["nc"] = _build_nc()
    return _CACHE["nc"]


def _q8(a):
    return np.clip(a, -240.0, 240.0).astype(E4)


def _prep_in_maps(inputs):
    f = lambda a: np.ascontiguousarray(np.asarray(a), dtype=np.float32)
    x, y = f(inputs["x"]), f(inputs["y"])
    W1, W2 = f(inputs["W1"]), f(inputs["W2"])

    def wq(W):
        # [HID, HID] -> quantized + residual, laid out [128, KB, HID]
        Ws = W * S_W
        q = _q8(Ws)
        r = _q8(Ws - q.astype(np.float32))
        re = lambda a: np.ascontiguousarray(
            a.reshape(KB, 128, HID).transpose(1, 0, 2)
        )
        return re(q), re(r)

    W1qa, W1ra = wq(W1)
    W2qa, W2ra = wq(W2)
    shared = {
        "yT": f(y.T),
        "Wxs": f(inputs["Wx"]) * np.float32(S_H),
        "Wys": f(inputs["Wy"]) * np.float32(S_H),
        "b0s": f(np.asarray(inputs["b0"]).reshape(KB, 128).T * S_H),
        "W1q": W1qa,
        "W1r": W1ra,
        "W2q": W2qa,
        "W2r": W2ra,
        "b1s": f(np.asarray(inputs["b1"]).reshape(KB, 128).T * S_H),
        "b2s": f(np.asarray(inputs["b2"]).reshape(KB, 128).T * (S_H * S_W)),
        "W3c": np.ascontiguousarray(
            np.repeat(
                f(np.asarray(inputs["W3"]).reshape(KB, 128).T)[:, :, None], 2, axis=2
            )
        ),
    }
    in_maps = []
    for m in range(N_CORES):
        im = dict(shared)
        im["xT"] = f(x[m * ROWS : (m + 1) * ROWS].T)
        in_maps.append(im)
    return in_maps


def run(trace=False, **inputs):
    nc = _get_nc()
    in_maps = _prep_in_maps(inputs)
    res = run_bass_kernel_spmd(nc, in_maps, core_ids=list(range(N_CORES)), trace=trace)
    b3 = np.float32(np.asarray(inputs["b3"]).reshape(-1)[0])
    inv = np.float32(1.0 / (S_H * S_W))
    blocks = []
    for r in res.results:
        o = r["out"]  # [128, 64]: col = 4*t + pb, partition = pair-in-block
        a = o.reshape(128, N_TILES, 2, 2)  # [p, t, a_half, j_half]
        a = a.transpose(1, 2, 3, 0)  # [t, a_half, j_half, p]
        blocks.append(a.reshape(ROWS, B) * inv + b3)
    return np.concatenate(blocks, axis=0).astype(np.float32), res


def kernel(**inputs):
    out, _ = run(trace=False, **inputs)
    return out
